# revision 3
# baseline (speedup 1.0000x reference)
"""Causal self-attention (B=2, S=2048, E=2048, H=16) on 8 TRN2 NeuronCores.

Sharding: 2-way batch x 4-way head-group tensor parallel.
Core c handles batch c//4 and heads [4*(c%4), 4*(c%4)+4).

All matmuls run in bf16 (1 cyc/row on the PE) with fp32 PSUM accumulation;
a numpy simulation of this exact rounding chain gives absmax-rel ~5.5e-3
vs the fp64 reference (gate is 2e-2).

Per-core kernel:
  inputs arrive pre-transposed/pre-scaled from the host:
    xt  = bf16(X^T)            [E, S]   (feature-major activations)
    wq  = bf16(W_q * SCALE)    [E, 512] (scale folded into W_q/b_q)
    wk  = bf16(W_k)            [E, 512] (b_k dropped: softmax shift-invariant)
    wv  = bf16(W_v)            [E, 512] (b_v folded into host-side out bias)
    wo  = bf16(W_out rows)     [512, E]
  phase 1: V projection, token-major:  vsb[i] = X_i @ Wv   [128 tok, 512]
  phase 2: per head h: q/k feature-major ([128 hd, S]), then causal
           attention: bf16 scores -> exp (fp32 PSUM in, bf16 out, fp32
           row-sum accum) -> P^T via PE matmul against diag(1/rowsum)
           (transpose + softmax normalize in one op) -> PV -> attT bf16
  phase 3: out projection from attT/wo, fp32 out [S, E]

Host: shard + bf16-cast inputs, run SPMD on 8 cores, sum the 4 head-group
partials per batch and add (b_out + b_v @ W_out) once.
"""

from contextlib import ExitStack

import ml_dtypes
import numpy as np

import concourse.bass as bass
import concourse.tile as tile
from concourse import bacc, bass_utils, mybir
from concourse.masks import make_causal_mask, make_identity

FP = mybir.dt.float32
BF = mybir.dt.bfloat16
AF = mybir.ActivationFunctionType

B = 2
S = 2048
E = 2048
H = 16
HD = 128
NCORES = 8
HG = 4  # head-group axis (tensor parallel)
H_LOC = H // HG  # 4 heads per core
FLOC = H_LOC * HD  # 512 local features per q/k/v
SCALE = 1.0 / float(np.sqrt(HD))
NEG = -1.0e30

PROFILE = False
LAST_EXEC_NS = None
LAST_RESULTS = None


def _emit(nc, S=S, E=E):
    NB = S // 128  # 16 token blocks
    EB = E // 128  # 16 feature blocks
    xt = nc.dram_tensor("xt", [E, S], BF, kind="ExternalInput").ap()
    wq = nc.dram_tensor("wq", [E, FLOC], BF, kind="ExternalInput").ap()
    wk = nc.dram_tensor("wk", [E, FLOC], BF, kind="ExternalInput").ap()
    wv = nc.dram_tensor("wv", [E, FLOC], BF, kind="ExternalInput").ap()
    bqs = nc.dram_tensor("bqs", [FLOC, 1], FP, kind="ExternalInput").ap()
    wo = nc.dram_tensor("wo", [FLOC, E], BF, kind="ExternalInput").ap()
    out = nc.dram_tensor("out", [S, E], FP, kind="ExternalOutput").ap()

    with tile.TileContext(nc) as tc, ExitStack() as top:
        cst = top.enter_context(tc.tile_pool(name="cst", bufs=1))
        ident = cst.tile([128, 128], FP, name="ident", tag="ident")
        make_identity(nc, ident[:])
        cmask = cst.tile([128, 128], FP, name="cmask", tag="cmask")
        make_causal_mask(nc, cmask[:], mask_val=NEG)
        bq_sb = cst.tile([128, H_LOC], FP, name="bq", tag="bq")
        for f in range(H_LOC):
            nc.sync.dma_start(bq_sb[:, f : f + 1], bqs[128 * f : 128 * (f + 1), :])

        # W_out rows live for the whole kernel; DMA'd up front.
        wo_pool = top.enter_context(tc.tile_pool(name="wo", bufs=1))
        wo_sb = []
        for h in range(H_LOC):
            wt = wo_pool.tile([128, E], BF, name=f"wo{h}", tag=f"wo{h}")
            nc.sync.dma_start(wt[:], wo[128 * h : 128 * (h + 1), :])
            wo_sb.append(wt)

        psA = top.enter_context(tc.tile_pool(name="psA", bufs=2, space="PSUM"))

        with ExitStack() as main:
            # X^T in bf16, streamed in by 512-token column chunks so the
            # first projections start after ~1/4 of X has landed.
            xts_pool = main.enter_context(tc.tile_pool(name="xts", bufs=1))
            xts = [
                xts_pool.tile([128, S], BF, name=f"xts{j}", tag=f"xts{j}")
                for j in range(EB)
            ]
            for sc in range(S // 512):
                csl = slice(512 * sc, 512 * (sc + 1))
                for j in range(EB):
                    nc.sync.dma_start(
                        xts[j][:, csl], xt[128 * j : 128 * (j + 1), csl]
                    )

            # ---------------- phase 1: V projection (token-major) ----------
            vsb_pool = main.enter_context(tc.tile_pool(name="vsb", bufs=1))
            vsb = [
                vsb_pool.tile([128, FLOC], BF, name=f"vsb{i}", tag=f"vsb{i}")
                for i in range(NB)
            ]
            with ExitStack() as vscope, nc.named_scope("vproj"):
                wv_pool = vscope.enter_context(tc.tile_pool(name="wv", bufs=1))
                wv_sb = []
                for e in range(EB):
                    wt = wv_pool.tile([128, FLOC], BF, name=f"wv{e}", tag=f"wv{e}")
                    nc.sync.dma_start(wt[:], wv[128 * e : 128 * (e + 1), :])
                    wv_sb.append(wt)
                for i in range(NB):
                    ps = psA.tile([128, FLOC], FP, name="ps", tag="ps")
                    for e in range(EB):
                        nc.tensor.matmul(
                            ps[:],
                            xts[e][:, 128 * i : 128 * (i + 1)],
                            wv_sb[e][:],
                            start=(e == 0),
                            stop=(e == EB - 1),
                        )
                    nc.scalar.activation(vsb[i][:], ps[:], AF.Copy)

            # ---------------- phase 2: per-head q/k projection + attention -
            qk_pool = main.enter_context(tc.tile_pool(name="qk", bufs=1))
            wqk_pool = main.enter_context(tc.tile_pool(name="wqk", bufs=2))
            att_pool = main.enter_context(tc.tile_pool(name="att", bufs=1))
            attT = [
                att_pool.tile([128, S], BF, name=f"attT{h}", tag=f"attT{h}")
                for h in range(H_LOC)
            ]
            p_pool = main.enter_context(tc.tile_pool(name="p", bufs=2))
            pt_pool = main.enter_context(tc.tile_pool(name="pt", bufs=2))
            rs_pool = main.enter_context(tc.tile_pool(name="rs", bufs=4))
            dg_pool = main.enter_context(tc.tile_pool(name="dg", bufs=2))
            ps_sc = main.enter_context(
                tc.tile_pool(name="ps_sc", bufs=2, space="PSUM")
            )
            ps_tp = main.enter_context(
                tc.tile_pool(name="ps_tp", bufs=2, space="PSUM")
            )
            ps_pv = main.enter_context(
                tc.tile_pool(name="ps_pv", bufs=2, space="PSUM")
            )

            for h in range(H_LOC):
                qt = qk_pool.tile([128, S], BF, name=f"qT{h}", tag=f"qT{h}")
                kt = qk_pool.tile([128, S], BF, name=f"kT{h}", tag=f"kT{h}")
                with nc.named_scope(f"qkproj{h}"):
                    for which, (wsrc, dstt) in enumerate(((wq, qt), (wk, kt))):
                        wt = wqk_pool.tile([128, E], BF, name="wqk", tag="wqk")
                        for e in range(EB):
                            nc.sync.dma_start(
                                wt[:, 128 * e : 128 * (e + 1)],
                                wsrc[
                                    128 * e : 128 * (e + 1),
                                    128 * h : 128 * (h + 1),
                                ],
                            )
                        for sc in range(S // 512):
                            csl = slice(512 * sc, 512 * (sc + 1))
                            ps = psA.tile([128, 512], FP, name="ps", tag="ps")
                            for e in range(EB):
                                nc.tensor.matmul(
                                    ps[:],
                                    wt[:, 128 * e : 128 * (e + 1)],
                                    xts[e][:, csl],
                                    start=(e == 0),
                                    stop=(e == EB - 1),
                                )
                            if which == 0:
                                nc.vector.tensor_scalar_add(
                                    dstt[:, csl], ps[:], bq_sb[:, h : h + 1]
                                )
                            else:
                                nc.scalar.activation(dstt[:, csl], ps[:], AF.Copy)

                with nc.named_scope(f"attn{h}"):
                    for g in range(S // 512):  # q-groups of 512
                        PT = pt_pool.tile([128, 4 * S], BF, name="PT", tag="PT")
                        nkc = 4 * (g + 1)  # 128-wide key blocks for this group
                        for qs in range(4):
                            i = 4 * g + qs  # q-block
                            L = 128 * (i + 1)
                            nq0 = 128 * i
                            p = p_pool.tile([128, S], BF, name="p", tag="p")
                            rs = rs_pool.tile([128, 8], FP, name="rs", tag="rs")
                            nch = (L + 511) // 512
                            for c in range(nch):
                                w = min(512, L - 512 * c)
                                psc = ps_sc.tile(
                                    [128, 512], FP, name="psc", tag="psc"
                                )
                                nc.tensor.matmul(
                                    psc[:, :w],
                                    qt[:, nq0 : nq0 + 128],
                                    kt[:, 512 * c : 512 * c + w],
                                    start=True,
                                    stop=True,
                                )
                                if c == nch - 1:
                                    nc.vector.tensor_add(
                                        psc[:, w - 128 : w],
                                        psc[:, w - 128 : w],
                                        cmask[:],
                                    )
                                nc.scalar.activation(
                                    p[:, 512 * c : 512 * c + w],
                                    psc[:, :w],
                                    AF.Exp,
                                    accum_out=rs[:, c : c + 1],
                                )
                            for c in range(1, nch):
                                nc.vector.tensor_add(
                                    rs[:, 0:1], rs[:, 0:1], rs[:, c : c + 1]
                                )
                            nc.vector.reciprocal(rs[:, 4:5], rs[:, 0:1])
                            # diag(1/rowsum): transpose+normalize in one matmul
                            dg = dg_pool.tile([128, 128], BF, name="dg", tag="dg")
                            nc.vector.tensor_scalar_mul(
                                dg[:], ident[:], rs[:, 4:5]
                            )
                            for jg in range((i + 1 + 3) // 4):
                                nm = min(4, i + 1 - 4 * jg)
                                ptp = ps_tp.tile(
                                    [128, 512], FP, name="ptp", tag="ptp"
                                )
                                for m in range(nm):
                                    j = 4 * jg + m
                                    nc.tensor.matmul(
                                        ptp[:, 128 * m : 128 * (m + 1)],
                                        p[:, 128 * j : 128 * (j + 1)],
                                        dg[:],
                                        start=True,
                                        stop=True,
                                    )
                                src = ptp[:, : 128 * nm].rearrange(
                                    "p (m q) -> p m q", q=128
                                )
                                dst = PT.rearrange("p (j q) -> p j q", q=512)[
                                    :, 4 * jg : 4 * jg + nm,
                                    128 * qs : 128 * (qs + 1),
                                ]
                                nc.scalar.activation(dst, src, AF.Copy)
                        # PV for the group
                        po = ps_pv.tile([128, 512], FP, name="po", tag="po")
                        for j in range(nkc):
                            qlo = max(0, 128 * (j - 4 * g))  # causal: q >= k
                            nc.tensor.matmul(
                                po[:, qlo:512],
                                vsb[j][:, 128 * h : 128 * (h + 1)],
                                PT[:, 512 * j + qlo : 512 * j + 512],
                                start=(j == 0),
                                stop=(j == nkc - 1),
                            )
                        gsl = slice(512 * g, 512 * (g + 1))
                        nc.scalar.activation(attT[h][:, gsl], po[:], AF.Copy)

        # ---------------- phase 3: output projection ----------------
        with ExitStack() as ph, nc.named_scope("outproj"):
            ostg = ph.enter_context(tc.tile_pool(name="ostg", bufs=4))
            for i in range(NB):
                for c in range(E // 512):
                    pso = psA.tile([128, 512], FP, name="ps", tag="ps")
                    for h in range(H_LOC):
                        nc.tensor.matmul(
                            pso[:],
                            attT[h][:, 128 * i : 128 * (i + 1)],
                            wo_sb[h][:, 512 * c : 512 * (c + 1)],
                            start=(h == 0),
                            stop=(h == H_LOC - 1),
                        )
                    ot = ostg.tile([128, 512], FP, name="ostg", tag="ostg")
                    nc.scalar.activation(ot[:], pso[:], AF.Copy)
                    nc.sync.dma_start(
                        out[128 * i : 128 * (i + 1), 512 * c : 512 * (c + 1)],
                        ot[:],
                    )


_NC_CACHE = None


def _get_nc():
    global _NC_CACHE
    if _NC_CACHE is None:
        nc = bacc.Bacc(
            "TRN2",
            target_bir_lowering=False,
            debug=False,
            num_devices=1,
            enable_asserts=False,
        )
        _emit(nc)
        nc.compile()
        _NC_CACHE = nc
    return _NC_CACHE


def _bf(a):
    return np.ascontiguousarray(a.astype(ml_dtypes.bfloat16))


def make_in_maps(inX, W_qkv, b_qkv, W_out):
    xts = [
        np.ascontiguousarray(inX[b].astype(ml_dtypes.bfloat16).T) for b in range(B)
    ]
    in_maps = []
    for c in range(NCORES):
        b = c // HG
        hg = c % HG
        sl = slice(FLOC * hg, FLOC * (hg + 1))
        in_maps.append(
            {
                "xt": xts[b],
                "wq": _bf(W_qkv[:, 0:E][:, sl] * SCALE),
                "wk": _bf(W_qkv[:, E : 2 * E][:, sl]),
                "wv": _bf(W_qkv[:, 2 * E : 3 * E][:, sl]),
                "bqs": np.ascontiguousarray(
                    (b_qkv[0:E][sl] * SCALE).reshape(FLOC, 1).astype(np.float32)
                ),
                "wo": _bf(W_out[sl, :]),
            }
        )
    return in_maps


def kernel(inX, W_qkv, b_qkv, W_out, b_out):
    global LAST_EXEC_NS, LAST_RESULTS
    inX = np.asarray(inX, dtype=np.float32)
    W_qkv = np.asarray(W_qkv, dtype=np.float32)
    b_qkv = np.asarray(b_qkv, dtype=np.float32)
    W_out = np.asarray(W_out, dtype=np.float32)
    b_out = np.asarray(b_out, dtype=np.float32)

    nc = _get_nc()
    in_maps = make_in_maps(inX, W_qkv, b_qkv, W_out)

    res = bass_utils.run_bass_kernel_spmd(
        nc, in_maps, core_ids=list(range(NCORES))
    )
    LAST_EXEC_NS = res.exec_time_ns
    LAST_RESULTS = res

    bias_full = (b_out + b_qkv[2 * E : 3 * E] @ W_out).astype(np.float32)
    out = np.empty((B, S, E), dtype=np.float32)
    for b in range(B):
        acc = res.results[HG * b + 0]["out"].astype(np.float64)
        for hg in range(1, HG):
            acc += res.results[HG * b + hg]["out"]
        out[b] = (acc + bias_full).astype(np.float32)
    return out


# revision 11
# speedup vs baseline: 4.2150x; 4.2150x over previous
"""Causal self-attention (B=2, S=2048, E=2048, H=16) on 8 TRN2 NeuronCores.

Sharding: 2-way batch x 4-way head-group tensor parallel.
Core c handles batch c//4 and heads [4*(c%4), 4*(c%4)+4).

All matmuls run in bf16 (1 cyc/row on the PE) with fp32 PSUM accumulation;
a numpy simulation of this exact rounding chain gives absmax-rel ~5.5e-3
vs the fp64 reference (gate is 2e-2).

Per-core kernel:
  inputs arrive pre-transposed/pre-scaled from the host:
    xt  = bf16(X^T)            [E, S]   (feature-major activations)
    wq  = bf16(W_q * SCALE)    [E, 512] (scale folded into W_q/b_q)
    wk  = bf16(W_k)            [E, 512] (b_k dropped: softmax shift-invariant)
    wv  = bf16(W_v)            [E, 512] (b_v folded into host-side out bias)
    wo  = bf16(W_out rows)     [512, E]
  phase 1: V projection, token-major:  vsb[i] = X_i @ Wv   [128 tok, 512]
  phase 2: per head h: q/k feature-major ([128 hd, S]), then causal
           attention: bf16 scores -> exp (fp32 PSUM in, bf16 out, fp32
           row-sum accum) -> P^T via PE matmul against diag(1/rowsum)
           (transpose + softmax normalize in one op) -> PV -> attT bf16
  phase 3: out projection from attT/wo, fp32 out [S, E]

Host: shard + bf16-cast inputs, run SPMD on 8 cores, sum the 4 head-group
partials per batch and add (b_out + b_v @ W_out) once.
"""

from contextlib import ExitStack

import ml_dtypes
import numpy as np

import concourse.bass as bass
import concourse.tile as tile
from concourse import bacc, bass_utils, mybir
from concourse.masks import make_causal_mask, make_identity

FP = mybir.dt.float32
BF = mybir.dt.bfloat16
AF = mybir.ActivationFunctionType

B = 2
S = 2048
E = 2048
H = 16
HD = 128
NCORES = 8
HG = 4  # head-group axis (tensor parallel)
H_LOC = H // HG  # 4 heads per core
FLOC = H_LOC * HD  # 512 local features per q/k/v
SCALE = 1.0 / float(np.sqrt(HD))
NEG = -1.0e30

PROFILE = False
LAST_EXEC_NS = None
LAST_RESULTS = None


def _emit(nc, reps=1, S=S, E=E):
    xt = nc.dram_tensor("xt", [E, S], BF, kind="ExternalInput").ap()
    wq = nc.dram_tensor("wq", [E, FLOC], BF, kind="ExternalInput").ap()
    wk = nc.dram_tensor("wk", [E, FLOC], BF, kind="ExternalInput").ap()
    wv = nc.dram_tensor("wv", [E, FLOC], BF, kind="ExternalInput").ap()
    bqs = nc.dram_tensor("bqs", [FLOC, 1], FP, kind="ExternalInput").ap()
    wo = nc.dram_tensor("wo", [FLOC, E], BF, kind="ExternalInput").ap()
    out = nc.dram_tensor("out", [S, E], FP, kind="ExternalOutput").ap()
    with tile.TileContext(nc) as tc:
        for _ in range(reps):
            _emit_body(nc, tc, xt, wq, wk, wv, bqs, wo, out, S, E)


def _emit_body(nc, tc, xt, wq, wk, wv, bqs, wo, out, S=S, E=E):
    NB = S // 128  # 16 token blocks
    EB = E // 128  # 16 feature blocks
    with ExitStack() as top:
        cst = top.enter_context(tc.tile_pool(name="cst", bufs=1))
        ident = cst.tile([128, 128], FP, name="ident", tag="ident")
        make_identity(nc, ident[:])
        cmask = cst.tile([128, 128], FP, name="cmask", tag="cmask")
        make_causal_mask(nc, cmask[:], mask_val=NEG)
        bq_sb = cst.tile([128, H_LOC], FP, name="bq", tag="bq")
        for f in range(H_LOC):
            nc.sync.dma_start(bq_sb[:, f : f + 1], bqs[128 * f : 128 * (f + 1), :])

        # W_out rows live for the whole kernel; the DMAs are emitted after
        # the attention loop so they queue behind the input-critical X/W
        # streams on the sync DMA queue.
        wo_pool = top.enter_context(tc.tile_pool(name="wo", bufs=1))
        wo_sb = [
            wo_pool.tile([128, E], BF, name=f"wo{h}", tag=f"wo{h}")
            for h in range(H_LOC)
        ]

        psA = top.enter_context(tc.tile_pool(name="psA", bufs=2, space="PSUM"))

        with ExitStack() as main:
            # X^T in bf16, streamed in by 512-token column chunks so the
            # first projections start after ~1/4 of X has landed.
            xts_pool = main.enter_context(tc.tile_pool(name="xts", bufs=1))
            xts = [
                xts_pool.tile([128, S], BF, name=f"xts{j}", tag=f"xts{j}")
                for j in range(EB)
            ]
            for sc in range(S // 512):
                csl = slice(512 * sc, 512 * (sc + 1))
                for j in range(EB):
                    nc.sync.dma_start(
                        xts[j][:, csl], xt[128 * j : 128 * (j + 1), csl]
                    )

            # ---------------- phase 1: V projection (token-major) ----------
            vsb_pool = main.enter_context(tc.tile_pool(name="vsb", bufs=1))
            vsb = [
                vsb_pool.tile([128, FLOC], BF, name=f"vsb{i}", tag=f"vsb{i}")
                for i in range(NB)
            ]
            with ExitStack() as vscope, nc.named_scope("vproj"):
                wv_pool = vscope.enter_context(tc.tile_pool(name="wv", bufs=1))
                wv_sb = []
                for e in range(EB):
                    wt = wv_pool.tile([128, FLOC], BF, name=f"wv{e}", tag=f"wv{e}")
                    nc.sync.dma_start(wt[:], wv[128 * e : 128 * (e + 1), :])
                    wv_sb.append(wt)
                for i in range(NB):
                    ps = psA.tile([128, FLOC], FP, name="ps", tag="ps")
                    for e in range(EB):
                        nc.tensor.matmul(
                            ps[:],
                            xts[e][:, 128 * i : 128 * (i + 1)],
                            wv_sb[e][:],
                            start=(e == 0),
                            stop=(e == EB - 1),
                        )
                    nc.scalar.activation(vsb[i][:], ps[:], AF.Copy)

            # ---------------- phase 2: per-head q/k projection + attention -
            qk_pool = main.enter_context(tc.tile_pool(name="qk", bufs=1))
            wqk_pool = main.enter_context(tc.tile_pool(name="wqk", bufs=2))
            att_pool = main.enter_context(tc.tile_pool(name="att", bufs=1))
            attT = [
                att_pool.tile([128, S], BF, name=f"attT{h}", tag=f"attT{h}")
                for h in range(H_LOC)
            ]
            p_pool = main.enter_context(tc.tile_pool(name="p", bufs=2))
            pt_pool = main.enter_context(tc.tile_pool(name="pt", bufs=2))
            rs_pool = main.enter_context(tc.tile_pool(name="rs", bufs=4))
            dg_pool = main.enter_context(tc.tile_pool(name="dg", bufs=2))
            ps_sc = main.enter_context(
                tc.tile_pool(name="ps_sc", bufs=3, space="PSUM")
            )
            ps_tp = main.enter_context(
                tc.tile_pool(name="ps_tp", bufs=2, space="PSUM")
            )
            ps_pv = main.enter_context(
                tc.tile_pool(name="ps_pv", bufs=1, space="PSUM")
            )

            for h in range(H_LOC):
                qt = qk_pool.tile([128, S], BF, name=f"qT{h}", tag=f"qT{h}")
                kt = qk_pool.tile([128, S], BF, name=f"kT{h}", tag=f"kT{h}")
                with nc.named_scope(f"qkproj{h}"):
                    for which, (wsrc, dstt) in enumerate(((wq, qt), (wk, kt))):
                        wt = wqk_pool.tile([128, E], BF, name="wqk", tag="wqk")
                        for e in range(EB):
                            nc.sync.dma_start(
                                wt[:, 128 * e : 128 * (e + 1)],
                                wsrc[
                                    128 * e : 128 * (e + 1),
                                    128 * h : 128 * (h + 1),
                                ],
                            )
                        for sc in range(S // 512):
                            csl = slice(512 * sc, 512 * (sc + 1))
                            ps = psA.tile([128, 512], FP, name="ps", tag="ps")
                            for e in range(EB):
                                nc.tensor.matmul(
                                    ps[:],
                                    wt[:, 128 * e : 128 * (e + 1)],
                                    xts[e][:, csl],
                                    start=(e == 0),
                                    stop=(e == EB - 1),
                                )
                            if which == 0:
                                nc.vector.tensor_scalar_add(
                                    dstt[:, csl], ps[:], bq_sb[:, h : h + 1]
                                )
                            else:
                                nc.scalar.activation(dstt[:, csl], ps[:], AF.Copy)

                with nc.named_scope(f"attn{h}"):
                    # Software-pipelined: scores/exp for q-block i are
                    # emitted before the P^T matmuls of q-block i-1, so the
                    # in-order PE queue never waits on the scalar-engine exp
                    # -> rowsum -> diag chain.
                    PTs = {}
                    ps = {}
                    dgs = {}

                    def scores(i):
                        L = 128 * (i + 1)
                        nq0 = 128 * i
                        p = p_pool.tile([128, S], BF, name="p", tag="p")
                        rs = rs_pool.tile([128, 8], FP, name="rs", tag="rs")
                        nch = (L + 511) // 512
                        for c in range(nch):
                            w = min(512, L - 512 * c)
                            psc = ps_sc.tile(
                                [128, 512], FP, name="psc", tag="psc"
                            )
                            nc.tensor.matmul(
                                psc[:, :w],
                                qt[:, nq0 : nq0 + 128],
                                kt[:, 512 * c : 512 * c + w],
                                start=True,
                                stop=True,
                            )
                            if c == nch - 1:
                                nc.vector.tensor_add(
                                    psc[:, w - 128 : w],
                                    psc[:, w - 128 : w],
                                    cmask[:],
                                )
                            nc.scalar.activation(
                                p[:, 512 * c : 512 * c + w],
                                psc[:, :w],
                                AF.Exp,
                                accum_out=rs[:, c : c + 1],
                            )
                        for c in range(1, nch):
                            nc.vector.tensor_add(
                                rs[:, 0:1], rs[:, 0:1], rs[:, c : c + 1]
                            )
                        nc.vector.reciprocal(rs[:, 4:5], rs[:, 0:1])
                        # diag(1/rowsum): transpose+normalize in one matmul
                        dg = dg_pool.tile([128, 128], BF, name="dg", tag="dg")
                        nc.vector.tensor_scalar_mul(dg[:], ident[:], rs[:, 4:5])
                        ps[i] = p
                        dgs[i] = dg

                    def ptrans(i):
                        g = i // 4
                        qs = i % 4
                        if qs == 0:
                            PTs[g] = pt_pool.tile(
                                [128, 4 * S], BF, name="PT", tag="PT"
                            )
                        PT = PTs[g]
                        p = ps.pop(i)
                        dg = dgs.pop(i)
                        for jg in range((i + 1 + 3) // 4):
                            nm = min(4, i + 1 - 4 * jg)
                            ptp = ps_tp.tile([128, 512], FP, name="ptp", tag="ptp")
                            for m in range(nm):
                                j = 4 * jg + m
                                nc.tensor.matmul(
                                    ptp[:, 128 * m : 128 * (m + 1)],
                                    p[:, 128 * j : 128 * (j + 1)],
                                    dg[:],
                                    start=True,
                                    stop=True,
                                )
                            src = ptp[:, : 128 * nm].rearrange(
                                "p (m q) -> p m q", q=128
                            )
                            dst = PT.rearrange("p (j q) -> p j q", q=512)[
                                :, 4 * jg : 4 * jg + nm,
                                128 * qs : 128 * (qs + 1),
                            ]
                            nc.vector.tensor_copy(dst, src)

                    def pv(g):
                        PT = PTs.pop(g)
                        nkc = 4 * (g + 1)
                        po = ps_pv.tile([128, 512], FP, name="po", tag="po")
                        for j in range(nkc):
                            qlo = max(0, 128 * (j - 4 * g))  # causal: q >= k
                            nc.tensor.matmul(
                                po[:, qlo:512],
                                vsb[j][:, 128 * h : 128 * (h + 1)],
                                PT[:, 512 * j + qlo : 512 * j + 512],
                                start=(j == 0),
                                stop=(j == nkc - 1),
                            )
                        gsl = slice(512 * g, 512 * (g + 1))
                        nc.scalar.activation(attT[h][:, gsl], po[:], AF.Copy)

                    for i in range(NB):
                        scores(i)
                        if i > 0:
                            ptrans(i - 1)
                            if (i - 1) % 4 == 3:
                                pv((i - 1) // 4)
                    ptrans(NB - 1)
                    pv(NB // 4 - 1)

            for h in range(H_LOC):
                nc.sync.dma_start(wo_sb[h][:], wo[128 * h : 128 * (h + 1), :])

        # ---------------- phase 3: output projection ----------------
        with ExitStack() as ph, nc.named_scope("outproj"):
            ostg = ph.enter_context(tc.tile_pool(name="ostg", bufs=4))
            for i in range(NB):
                for c in range(E // 512):
                    pso = psA.tile([128, 512], FP, name="ps", tag="ps")
                    for h in range(H_LOC):
                        nc.tensor.matmul(
                            pso[:],
                            attT[h][:, 128 * i : 128 * (i + 1)],
                            wo_sb[h][:, 512 * c : 512 * (c + 1)],
                            start=(h == 0),
                            stop=(h == H_LOC - 1),
                        )
                    ot = ostg.tile([128, 512], FP, name="ostg", tag="ostg")
                    nc.scalar.activation(ot[:], pso[:], AF.Copy)
                    # Activation-engine DMA queue: keeps the 16MB output
                    # stream off the sync queue that feeds next-use inputs.
                    nc.scalar.dma_start(
                        out[128 * i : 128 * (i + 1), 512 * c : 512 * (c + 1)],
                        ot[:],
                    )


_NC_CACHE = {}


def _get_nc(reps=1):
    if reps not in _NC_CACHE:
        nc = bacc.Bacc(
            "TRN2",
            target_bir_lowering=False,
            debug=False,
            num_devices=1,
            enable_asserts=False,
        )
        _emit(nc, reps=reps)
        nc.compile()
        _NC_CACHE[reps] = nc
    return _NC_CACHE[reps]


def _bf(a):
    return np.ascontiguousarray(a.astype(ml_dtypes.bfloat16))


def make_in_maps(inX, W_qkv, b_qkv, W_out):
    xts = [
        np.ascontiguousarray(inX[b].astype(ml_dtypes.bfloat16).T) for b in range(B)
    ]
    in_maps = []
    for c in range(NCORES):
        b = c // HG
        hg = c % HG
        sl = slice(FLOC * hg, FLOC * (hg + 1))
        in_maps.append(
            {
                "xt": xts[b],
                "wq": _bf(W_qkv[:, 0:E][:, sl] * SCALE),
                "wk": _bf(W_qkv[:, E : 2 * E][:, sl]),
                "wv": _bf(W_qkv[:, 2 * E : 3 * E][:, sl]),
                "bqs": np.ascontiguousarray(
                    (b_qkv[0:E][sl] * SCALE).reshape(FLOC, 1).astype(np.float32)
                ),
                "wo": _bf(W_out[sl, :]),
            }
        )
    return in_maps


def kernel(inX, W_qkv, b_qkv, W_out, b_out):
    global LAST_EXEC_NS, LAST_RESULTS
    inX = np.asarray(inX, dtype=np.float32)
    W_qkv = np.asarray(W_qkv, dtype=np.float32)
    b_qkv = np.asarray(b_qkv, dtype=np.float32)
    W_out = np.asarray(W_out, dtype=np.float32)
    b_out = np.asarray(b_out, dtype=np.float32)

    nc = _get_nc()
    in_maps = make_in_maps(inX, W_qkv, b_qkv, W_out)

    res = bass_utils.run_bass_kernel_spmd(
        nc, in_maps, core_ids=list(range(NCORES))
    )
    LAST_EXEC_NS = res.exec_time_ns
    LAST_RESULTS = res

    bias_full = (b_out + b_qkv[2 * E : 3 * E] @ W_out).astype(np.float32)
    out = np.empty((B, S, E), dtype=np.float32)
    for b in range(B):
        acc = res.results[HG * b + 0]["out"].astype(np.float64)
        for hg in range(1, HG):
            acc += res.results[HG * b + hg]["out"]
        out[b] = (acc + bias_full).astype(np.float32)
    return out
